# revision 26
# baseline (speedup 1.0000x reference)
"""Trainium2 Bass kernel for additive (Bahdanau-style) attention.

reference math (B=4, Tq=Tp=512, D=256):
    prod_q = q @ W0                                   [B,Tq,D]
    prod_p = p @ W1                                   [B,Tp,D]
    scores[b,p,q] = sum_e vc[e] * tanh(prod_p[b,p,e] + prod_q[b,q,e])
    weights = softmax(scores, axis=p)
    out[b,p,d] = sum_q weights[b,p,q] * q[b,q,d]

Sharding: 8 cores; core c handles batch b = c//2 and p-rows
[256*(c%2), 256*(c%2)+256).  The softmax denominator (per (b,q)) needs the
exp-sum over all p, so the two cores sharing a batch AllReduce a 512-float
vector; everything else is core-local.

Per-core layout: e (=D) lives on SBUF partitions (2 halves of 128).  The
broadcast add prod_p[:,p] + prod_q is a DVE tensor_scalar with a
per-partition scalar (fp32, 2x mode); tanh runs as one big ACT
instruction per p-block (fp16 out); the vc contraction is a PE matmul
(lhsT = tanh tile [e,q-chunk] fp16, rhs = vc [e,1]) accumulating score
columns S^T[q,p] in PSUM, which makes the softmax a free-axis op and
feeds the final matmul out = E^T @ (q/Z).

The kernel is ACT-bound: 256 p x 512 q x 256 e = 33.5M tanh per core at
128 lanes x 1.2 GHz = 218.5 us floor.  Cost-model timeline: ~259.5 us
per core (ACT 87% busy, gap-free through the main loop), plus the real
pairwise AllReduce (~10-20 us, not modeled).  Measured end-to-end
relative error vs the fp32 reference: 1.8e-4.
"""

import sys

if "/opt/trn_rl_repo" not in sys.path:
    sys.path.insert(0, "/opt/trn_rl_repo")

import numpy as np

B, TQ, TP, D = 4, 512, 512, 256
N_CORES = 8
PHALF = TP // 2  # p-rows per core
PBLK = 10        # p-rows per inner block
NBLK = 32
P = 128          # SBUF partitions

_cache = {}


def _build(bench_mode=False, n_blocks=NBLK):
    import concourse.bacc as bacc
    import concourse.tile as tile
    from concourse import mybir

    f32 = mybir.dt.float32
    f16 = mybir.dt.float16
    Alu = mybir.AluOpType
    Act = mybir.ActivationFunctionType

    nc = bacc.Bacc(
        "TRN2", target_bir_lowering=False, debug=False,
        num_devices=1 if bench_mode else N_CORES,
    )

    qb = nc.dram_tensor("qb", [TQ, D], f32, kind="ExternalInput")
    pb = nc.dram_tensor("pb", [PHALF, D], f32, kind="ExternalInput")
    w0 = nc.dram_tensor("W0", [D, D], f32, kind="ExternalInput")
    w1 = nc.dram_tensor("W1", [D, D], f32, kind="ExternalInput")
    vc = nc.dram_tensor("vc", [D, 1], f32, kind="ExternalInput")
    eye = nc.dram_tensor("eye", [P, P], f32, kind="ExternalInput")
    y = nc.dram_tensor("y", [PHALF, D], f32, kind="ExternalOutput")

    NQC = TQ // P   # 4 q chunks
    NDC = D // P    # 2 d/e chunks
    NPC = PHALF // P  # 2 p chunks

    with tile.TileContext(nc) as tc:
        with (
            tc.tile_pool(name="const", bufs=1) as cp,
            tc.tile_pool(name="ein", bufs=2) as einp,
            tc.tile_pool(name="tt", bufs=2) as ttp,
            tc.tile_pool(name="ps_misc", bufs=1, space="PSUM") as psm,
            tc.tile_pool(name="ps_st", bufs=1, space="PSUM") as psst,
            tc.tile_pool(name="dram", bufs=1, space="DRAM") as dramp,
        ):
            eyesb = cp.tile([P, P], f32, tag="eye")
            nc.sync.dma_start(eyesb[:], eye[:])

            # input DMAs: one consolidated transfer per tensor (issue cost
            # on the DMA queues dominates, so fewer+larger is better), spread
            # over the two queues in critical-path order
            qn = cp.tile([P, NQC, D], f32, tag="qn")
            nc.sync.dma_start(
                qn[:], qb.rearrange("(c p) d -> p c d", p=P)
            )
            qn32 = [qn[:, qc, :] for qc in range(NQC)]
            w0t = cp.tile([P, NDC, D], f32, tag="w0t")
            nc.gpsimd.dma_start(
                w0t[:], w0.rearrange("(c p) d -> p c d", p=P)
            )
            w0sb = [[w0t[:, dc, h * P : (h + 1) * P] for h in range(NDC)]
                    for dc in range(NDC)]
            pn = cp.tile([P, NPC, D], f32, tag="pn")
            nc.sync.dma_start(
                pn[:], pb.rearrange("(c p) d -> p c d", p=P)
            )
            pn32 = [pn[:, pc, :] for pc in range(NPC)]
            w1t = cp.tile([P, NDC, D], f32, tag="w1t")
            nc.gpsimd.dma_start(
                w1t[:], w1.rearrange("(c p) d -> p c d", p=P)
            )
            w1sb = [[w1t[:, dc, h * P : (h + 1) * P] for h in range(NDC)]
                    for dc in range(NDC)]
            vct = cp.tile([P, NDC], f32, tag="vct")
            nc.sync.dma_start(vct[:], vc.rearrange("(c p) o -> p (c o)", p=P))
            dma_engines = [nc.sync, nc.gpsimd]

            vcbf = []
            for h in range(NDC):
                tb = cp.tile([P, 1], f16, tag=f"vcbf_{h}")
                nc.vector.tensor_copy(tb[:], vct[:, h : h + 1])
                vcbf.append(tb)

            qnf16 = []
            for qc in range(NQC):
                t = cp.tile([P, D], f16, tag=f"qnf16_{qc}", name=f"qnf16_{qc}")
                nc.vector.tensor_copy(t[:], qn32[qc][:])
                qnf16.append(t)

            # PE transposes: qT[d, q] and pT[d, p] (fp32)
            qT = [cp.tile([P, TQ], f32, tag=f"qT_{dc}", name=f"qT_{dc}") for dc in range(NDC)]
            pT = [cp.tile([P, PHALF], f32, tag=f"pT_{dc}", name=f"pT_{dc}") for dc in range(NDC)]
            for dc in range(NDC):
                for qc in range(NQC):
                    ps = psm.tile([P, P], f32, tag="tp", name="ps", bufs=3)
                    nc.tensor.transpose(
                        ps[:], qn32[qc][:, dc * P : (dc + 1) * P], eyesb[:]
                    )
                    nc.vector.tensor_copy(qT[dc][:, qc * P : (qc + 1) * P], ps[:])
                for pc in range(NPC):
                    ps = psm.tile([P, P], f32, tag="tp", name="ps", bufs=3)
                    nc.tensor.transpose(
                        ps[:], pn32[pc][:, dc * P : (dc + 1) * P], eyesb[:]
                    )
                    nc.vector.tensor_copy(pT[dc][:, pc * P : (pc + 1) * P], ps[:])

            # prod_qT[e, q] = (q @ W0)^T  and  prod_pT[e, p] = (p @ W1)^T (fp32)
            pq = [cp.tile([P, TQ], f32, tag=f"pq_{h}", name=f"pq_{h}") for h in range(NDC)]
            pp = [cp.tile([P, PHALF], f32, tag=f"pp_{h}", name=f"pp_{h}") for h in range(NDC)]

            def emit_prods(h):
                ps = psm.tile([P, TQ], f32, tag="prod", name="ps")
                for dc in range(NDC):
                    nc.tensor.matmul(
                        ps[:], w0sb[dc][h][:], qT[dc][:],
                        start=(dc == 0), stop=(dc == NDC - 1),
                    )
                nc.vector.tensor_copy(pq[h][:], ps[:])
                ps2 = psm.tile([P, PHALF], f32, tag="prod", name="ps2")
                for dc in range(NDC):
                    nc.tensor.matmul(
                        ps2[:], w1sb[dc][h][:], pT[dc][:],
                        start=(dc == 0), stop=(dc == NDC - 1),
                    )
                nc.vector.tensor_copy(pp[h][:], ps2[:])

            # score accumulators S^T[q, p] in PSUM (fp32), one per q-chunk
            st = [psst.tile([P, PHALF], f32, tag=f"st_{qc}", name=f"st_{qc}") for qc in range(NQC)]

            # ---- main loop over p blocks ----
            # ramp-in: small h-split blocks, emitted h=0-first so the first
            # tanh only waits on the h=0 prods; then steady blocks of PBLK
            def emit_vc_matmuls(tt_ap, base_off, p0, cnt, h_list):
                for j in range(cnt):
                    pidx = p0 + j
                    for qc in range(NQC):
                        for h in h_list:
                            off = base_off(h) + j * TQ + qc * P
                            nc.tensor.matmul(
                                st[qc][:, pidx : pidx + 1],
                                tt_ap[:, off : off + P],
                                vcbf[h][:],
                                start=(h == 0),
                                stop=(h == NDC - 1),
                                skip_group_check=True,
                            )

            def emit_ramp_half(p0, cnt, h):
                # tanh for one e-half of a ramp block; matmuls are emitted
                # later (per-column h0/h1 adjacency keeps PSUM has_written
                # accumulation valid: each column's start=True must
                # immediately precede its stop=True partner on the bank)
                w = cnt * TQ
                ein = einp.tile(
                    [P, w], f32, tag=f"ein_r{p0}", name="ein", bufs=1
                )
                for j in range(cnt):
                    nc.vector.tensor_scalar(
                        ein[:, j * TQ : (j + 1) * TQ],
                        pq[h][:],
                        pp[h][:, p0 + j : p0 + j + 1],
                        None,
                        Alu.add,
                    )
                tth = ttp.tile(
                    [P, w], f16, tag=f"tt_r{p0}_{h}", name="tt", bufs=1
                )
                nc.scalar.activation(tth[:], ein[:], Act.Tanh)
                return tth

            def emit_ramp_matmuls(p0, cnt, tths):
                for j in range(cnt):
                    pidx = p0 + j
                    for qc in range(NQC):
                        for h in range(NDC):
                            off = j * TQ + qc * P
                            nc.tensor.matmul(
                                st[qc][:, pidx : pidx + 1],
                                tths[h][:, off : off + P],
                                vcbf[h][:],
                                start=(h == 0),
                                stop=(h == NDC - 1),
                                skip_group_check=True,
                            )

            def emit_block(p0, cnt):
                w = cnt * TQ
                ein = einp.tile([P, 2 * w], f32, tag="ein", name="ein")
                for h in range(NDC):
                    for j in range(cnt):
                        nc.vector.tensor_scalar(
                            ein[:, h * w + j * TQ : h * w + (j + 1) * TQ],
                            pq[h][:],
                            pp[h][:, p0 + j : p0 + j + 1],
                            None,
                            Alu.add,
                        )
                tt = ttp.tile([P, 2 * w], f16, tag="tt", name="tt")
                nc.scalar.activation(tt[:], ein[:], Act.Tanh)
                emit_vc_matmuls(tt, lambda h: h * w, p0, cnt, list(range(NDC)))

            n_rows = PHALF if n_blocks == NBLK else n_blocks * 8
            ramp = [(0, 2), (2, 6)]
            ramp_tts = {}
            emit_prods(0)
            for p0, cnt in ramp:
                ramp_tts[p0] = [emit_ramp_half(p0, cnt, 0)]
            emit_prods(1)
            for p0, cnt in ramp:
                ramp_tts[p0].append(emit_ramp_half(p0, cnt, 1))
                emit_ramp_matmuls(p0, cnt, ramp_tts[p0])
            # first steady block is smaller so its adds finish sooner after
            # the ramp; the rest are PBLK rows
            p0 = 8
            if n_rows - p0 >= 8 and (n_rows - p0 - 8) % PBLK == 0:
                emit_block(p0, 8)
                p0 += 8
            full, last = divmod(n_rows - p0, PBLK)
            for _ in range(full):
                emit_block(p0, PBLK)
                p0 += PBLK
            if last:
                emit_block(p0, last)

            # ---- softmax over p (denominator shared across the core pair) ----
            et = [cp.tile([P, PHALF], f32, tag=f"et_{qc}", name=f"et_{qc}") for qc in range(NQC)]
            zl = cp.tile([P, NQC], f32, tag="zl")
            for qc in range(NQC):
                nc.scalar.activation(et[qc][:], st[qc][:], Act.Exp)
                nc.vector.tensor_reduce(
                    zl[:, qc : qc + 1], et[qc][:], mybir.AxisListType.X, Alu.add
                )

            zin = dramp.tile([P, NQC], f32)
            zout = dramp.tile([P, NQC], f32)
            nc.sync.dma_start(zin[:], zl[:])
            if bench_mode:
                nc.sync.dma_start(zout[:], zin[:])
            else:
                nc.gpsimd.collective_compute(
                    "AllReduce",
                    mybir.AluOpType.add,
                    replica_groups=[[0, 1], [2, 3], [4, 5], [6, 7]],
                    ins=[zin.opt()],
                    outs=[zout.opt()],
                )

            zg = cp.tile([P, NQC], f32, tag="zg")
            nc.sync.dma_start(zg[:], zout[:])
            rz = cp.tile([P, NQC], f32, tag="rz")
            nc.vector.reciprocal(rz[:], zg[:])
            ets = [cp.tile([P, PHALF], f16, tag=f"ets_{qc}", name=f"ets_{qc}") for qc in range(NQC)]
            for qc in range(NQC):
                nc.vector.tensor_scalar(
                    ets[qc][:], et[qc][:], rz[:, qc : qc + 1], None, Alu.mult
                )

            # ---- out[p, d] = sum_q (E/Z)[q, p] * q[q, d] ----
            for mc in range(NPC):
                ops = psm.tile([P, D], f32, tag="tp", name="ops", bufs=3)
                for qc in range(NQC):
                    nc.tensor.matmul(
                        ops[:],
                        ets[qc][:, mc * P : (mc + 1) * P],
                        qnf16[qc][:],
                        start=(qc == 0),
                        stop=(qc == NQC - 1),
                    )
                osb = cp.tile([P, D], f32, tag=f"osb_{mc}")
                nc.vector.tensor_copy(osb[:], ops[:])
                dma_engines[mc % 2].dma_start(y[mc * P : (mc + 1) * P, :], osb[:])

    nc.compile()
    return nc


def _get_nc():
    if "nc" not in _cache:
        _cache["nc"] = _build()
    return _cache["nc"]


def kernel(q, p, W0, W1, vc, _trace=False, _trace_kwargs=None):
    q = np.ascontiguousarray(q, dtype=np.float32)
    p = np.ascontiguousarray(p, dtype=np.float32)
    W0 = np.ascontiguousarray(W0, dtype=np.float32)
    W1 = np.ascontiguousarray(W1, dtype=np.float32)
    vc = np.ascontiguousarray(vc, dtype=np.float32)
    eye = np.eye(P, dtype=np.float32)

    nc = _get_nc()
    from concourse.bass_utils import run_bass_kernel_spmd

    in_maps = []
    for c in range(N_CORES):
        b = c // 2
        p0 = PHALF * (c % 2)
        in_maps.append(
            {
                "qb": q[b],
                "pb": np.ascontiguousarray(p[b, p0 : p0 + PHALF]),
                "W0": W0,
                "W1": W1,
                "vc": vc,
                "eye": eye,
            }
        )

    kw = {}
    if _trace:
        kw["trace"] = True
        kw.update(_trace_kwargs or {})
    # the axon tunnel occasionally drops with a transient UNAVAILABLE
    # ("worker hung up"); retry a few times before giving up
    last_exc = None
    for attempt in range(4):
        try:
            res = run_bass_kernel_spmd(nc, in_maps, list(range(N_CORES)), **kw)
            break
        except Exception as e:  # noqa: BLE001
            last_exc = e
            if attempt == 3:
                raise
            import time as _time

            _time.sleep(5 * (attempt + 1))

    out = np.empty((B, TP, D), dtype=np.float32)
    for c in range(N_CORES):
        b = c // 2
        p0 = PHALF * (c % 2)
        out[b, p0 : p0 + PHALF] = res.results[c]["y"]

    if _trace:
        _cache["last_result"] = res
    return out


# revision 28
# speedup vs baseline: 1.0120x; 1.0120x over previous
"""Trainium2 Bass kernel for additive (Bahdanau-style) attention.

reference math (B=4, Tq=Tp=512, D=256):
    prod_q = q @ W0                                   [B,Tq,D]
    prod_p = p @ W1                                   [B,Tp,D]
    scores[b,p,q] = sum_e vc[e] * tanh(prod_p[b,p,e] + prod_q[b,q,e])
    weights = softmax(scores, axis=p)
    out[b,p,d] = sum_q weights[b,p,q] * q[b,q,d]

Sharding: 8 cores; core c handles batch b = c//2 and p-rows
[256*(c%2), 256*(c%2)+256).  The softmax denominator (per (b,q)) needs the
exp-sum over all p, so the two cores sharing a batch AllReduce a 512-float
vector; everything else is core-local.

Per-core layout: e (=D) lives on SBUF partitions (2 halves of 128).  The
broadcast add prod_p[:,p] + prod_q is a DVE tensor_scalar with a
per-partition scalar (fp32, 2x mode); tanh runs as one big ACT
instruction per p-block (fp16 out); the vc contraction is a PE matmul
(lhsT = tanh tile [e,q-chunk] fp16, rhs = vc [e,1]) accumulating score
columns S^T[q,p] in PSUM, which makes the softmax a free-axis op and
feeds the final matmul out = E^T @ (q/Z).

The kernel is ACT-bound: 256 p x 512 q x 256 e = 33.5M tanh per core at
128 lanes x 1.2 GHz = 218.5 us floor.  Cost-model timeline: ~259.5 us
per core (ACT 87% busy, gap-free through the main loop), plus the real
pairwise AllReduce (~10-20 us, not modeled).  Measured end-to-end
relative error vs the fp32 reference: 1.8e-4.
"""

import sys

if "/opt/trn_rl_repo" not in sys.path:
    sys.path.insert(0, "/opt/trn_rl_repo")

import numpy as np

B, TQ, TP, D = 4, 512, 512, 256
N_CORES = 8
PHALF = TP // 2  # p-rows per core
PBLK = 10        # p-rows per inner block
NBLK = 32
P = 128          # SBUF partitions

_cache = {}


def _build(bench_mode=False, n_blocks=NBLK):
    import concourse.bacc as bacc
    import concourse.tile as tile
    from concourse import mybir

    f32 = mybir.dt.float32
    f16 = mybir.dt.float16
    Alu = mybir.AluOpType
    Act = mybir.ActivationFunctionType

    nc = bacc.Bacc(
        "TRN2", target_bir_lowering=False, debug=False,
        num_devices=1 if bench_mode else N_CORES,
    )

    qb = nc.dram_tensor("qb", [TQ, D], f32, kind="ExternalInput")
    pb = nc.dram_tensor("pb", [PHALF, D], f32, kind="ExternalInput")
    w0 = nc.dram_tensor("W0", [D, D], f32, kind="ExternalInput")
    w1 = nc.dram_tensor("W1", [D, D], f32, kind="ExternalInput")
    vc = nc.dram_tensor("vc", [D, 1], f32, kind="ExternalInput")
    eye = nc.dram_tensor("eye", [P, P], f32, kind="ExternalInput")
    y = nc.dram_tensor("y", [PHALF, D], f32, kind="ExternalOutput")

    NQC = TQ // P   # 4 q chunks
    NDC = D // P    # 2 d/e chunks
    NPC = PHALF // P  # 2 p chunks

    with tile.TileContext(nc) as tc:
        with (
            tc.tile_pool(name="const", bufs=1) as cp,
            tc.tile_pool(name="ein", bufs=2) as einp,
            tc.tile_pool(name="tt", bufs=2) as ttp,
            tc.tile_pool(name="ps_misc", bufs=1, space="PSUM") as psm,
            tc.tile_pool(name="ps_st", bufs=1, space="PSUM") as psst,
            tc.tile_pool(name="dram", bufs=1, space="DRAM") as dramp,
        ):
            # input DMAs: one consolidated transfer per tensor (issue cost
            # on the DMA queues dominates, so fewer+larger is better), spread
            # over the two queues; qb first, it heads the critical path
            qn = cp.tile([P, NQC, D], f32, tag="qn")
            nc.sync.dma_start(
                qn[:], qb.rearrange("(c p) d -> p c d", p=P)
            )
            qn32 = [qn[:, qc, :] for qc in range(NQC)]
            w0t = cp.tile([P, NDC, D], f32, tag="w0t")
            nc.gpsimd.dma_start(
                w0t[:], w0.rearrange("(c p) d -> p c d", p=P)
            )
            eyesb = cp.tile([P, P], f32, tag="eye")
            nc.sync.dma_start(eyesb[:], eye[:])
            pn = cp.tile([P, NPC, D], f32, tag="pn")
            nc.sync.dma_start(
                pn[:], pb.rearrange("(c p) d -> p c d", p=P)
            )
            pn32 = [pn[:, pc, :] for pc in range(NPC)]
            w1t = cp.tile([P, NDC, D], f32, tag="w1t")
            nc.gpsimd.dma_start(
                w1t[:], w1.rearrange("(c p) d -> p c d", p=P)
            )
            vct = cp.tile([P, NDC], f32, tag="vct")
            nc.gpsimd.dma_start(vct[:], vc.rearrange("(c p) o -> p (c o)", p=P))
            dma_engines = [nc.sync, nc.gpsimd]

            vcbf = []
            for h in range(NDC):
                tb = cp.tile([P, 1], f16, tag=f"vcbf_{h}")
                nc.vector.tensor_copy(tb[:], vct[:, h : h + 1])
                vcbf.append(tb)

            qn16 = cp.tile([P, NQC, D], f16, tag="qn16")
            nc.vector.tensor_copy(qn16[:], qn[:])
            qnf16 = [qn16[:, qc, :] for qc in range(NQC)]
            pn16 = cp.tile([P, NPC, D], f16, tag="pn16")
            nc.vector.tensor_copy(pn16[:], pn[:])
            pn32 = [pn16[:, pc, :] for pc in range(NPC)]
            qn32 = qnf16
            w0t16 = cp.tile([P, NDC, D], f16, tag="w0t16")
            nc.vector.tensor_copy(w0t16[:], w0t[:])
            w0sb = [[w0t16[:, dc, h * P : (h + 1) * P] for h in range(NDC)]
                    for dc in range(NDC)]
            w1t16 = cp.tile([P, NDC, D], f16, tag="w1t16")
            nc.vector.tensor_copy(w1t16[:], w1t[:])
            w1sb = [[w1t16[:, dc, h * P : (h + 1) * P] for h in range(NDC)]
                    for dc in range(NDC)]
            eye16 = cp.tile([P, P], f16, tag="eye16")
            nc.vector.tensor_copy(eye16[:], eyesb[:])

            # PE transposes: qT[d, q] and pT[d, p] (fp32)
            qT = [cp.tile([P, TQ], f16, tag=f"qT_{dc}", name=f"qT_{dc}") for dc in range(NDC)]
            pT = [cp.tile([P, PHALF], f16, tag=f"pT_{dc}", name=f"pT_{dc}") for dc in range(NDC)]
            for dc in range(NDC):
                for qc in range(NQC):
                    ps = psm.tile([P, P], f16, tag="tpT", name="ps", bufs=2)
                    nc.tensor.transpose(
                        ps[:], qn32[qc][:, dc * P : (dc + 1) * P], eye16[:]
                    )
                    nc.vector.tensor_copy(qT[dc][:, qc * P : (qc + 1) * P], ps[:])
                for pc in range(NPC):
                    ps = psm.tile([P, P], f16, tag="tpT", name="ps", bufs=2)
                    nc.tensor.transpose(
                        ps[:], pn32[pc][:, dc * P : (dc + 1) * P], eye16[:]
                    )
                    nc.vector.tensor_copy(pT[dc][:, pc * P : (pc + 1) * P], ps[:])

            # prod_qT[e, q] = (q @ W0)^T  and  prod_pT[e, p] = (p @ W1)^T (fp32)
            pq = [cp.tile([P, TQ], f32, tag=f"pq_{h}", name=f"pq_{h}") for h in range(NDC)]
            pp = [cp.tile([P, PHALF], f32, tag=f"pp_{h}", name=f"pp_{h}") for h in range(NDC)]

            def emit_prods(h):
                ps = psm.tile([P, TQ], f32, tag="prod", name="ps", bufs=2)
                for dc in range(NDC):
                    nc.tensor.matmul(
                        ps[:], w0sb[dc][h][:], qT[dc][:],
                        start=(dc == 0), stop=(dc == NDC - 1),
                    )
                nc.vector.tensor_copy(pq[h][:], ps[:])
                ps2 = psm.tile([P, PHALF], f32, tag="prod", name="ps2", bufs=2)
                for dc in range(NDC):
                    nc.tensor.matmul(
                        ps2[:], w1sb[dc][h][:], pT[dc][:],
                        start=(dc == 0), stop=(dc == NDC - 1),
                    )
                nc.vector.tensor_copy(pp[h][:], ps2[:])

            # score accumulators S^T[q, p] in PSUM (fp32), one per q-chunk
            st = [psst.tile([P, PHALF], f32, tag=f"st_{qc}", name=f"st_{qc}") for qc in range(NQC)]

            # ---- main loop over p blocks ----
            # ramp-in: small h-split blocks, emitted h=0-first so the first
            # tanh only waits on the h=0 prods; then steady blocks of PBLK
            def emit_vc_matmuls(tt_ap, base_off, p0, cnt, h_list):
                for j in range(cnt):
                    pidx = p0 + j
                    for qc in range(NQC):
                        for h in h_list:
                            off = base_off(h) + j * TQ + qc * P
                            nc.tensor.matmul(
                                st[qc][:, pidx : pidx + 1],
                                tt_ap[:, off : off + P],
                                vcbf[h][:],
                                start=(h == 0),
                                stop=(h == NDC - 1),
                                skip_group_check=True,
                            )

            def emit_ramp_half(p0, cnt, h):
                # tanh for one e-half of a ramp block; matmuls are emitted
                # later (per-column h0/h1 adjacency keeps PSUM has_written
                # accumulation valid: each column's start=True must
                # immediately precede its stop=True partner on the bank)
                w = cnt * TQ
                ein = einp.tile(
                    [P, w], f32, tag=f"ein_r{p0}", name="ein", bufs=1
                )
                for j in range(cnt):
                    nc.vector.tensor_scalar(
                        ein[:, j * TQ : (j + 1) * TQ],
                        pq[h][:],
                        pp[h][:, p0 + j : p0 + j + 1],
                        None,
                        Alu.add,
                    )
                tth = ttp.tile(
                    [P, w], f16, tag=f"tt_r{p0}_{h}", name="tt", bufs=1
                )
                nc.scalar.activation(tth[:], ein[:], Act.Tanh)
                return tth

            def emit_ramp_matmuls(p0, cnt, tths):
                for j in range(cnt):
                    pidx = p0 + j
                    for qc in range(NQC):
                        for h in range(NDC):
                            off = j * TQ + qc * P
                            nc.tensor.matmul(
                                st[qc][:, pidx : pidx + 1],
                                tths[h][:, off : off + P],
                                vcbf[h][:],
                                start=(h == 0),
                                stop=(h == NDC - 1),
                                skip_group_check=True,
                            )

            def emit_block(p0, cnt):
                w = cnt * TQ
                ein = einp.tile([P, 2 * w], f32, tag="ein", name="ein")
                for h in range(NDC):
                    for j in range(cnt):
                        nc.vector.tensor_scalar(
                            ein[:, h * w + j * TQ : h * w + (j + 1) * TQ],
                            pq[h][:],
                            pp[h][:, p0 + j : p0 + j + 1],
                            None,
                            Alu.add,
                        )
                tt = ttp.tile([P, 2 * w], f16, tag="tt", name="tt")
                nc.scalar.activation(tt[:], ein[:], Act.Tanh)
                emit_vc_matmuls(tt, lambda h: h * w, p0, cnt, list(range(NDC)))

            n_rows = PHALF if n_blocks == NBLK else n_blocks * 8
            ramp = [(0, 2), (2, 6)]
            ramp_tts = {}
            emit_prods(0)
            for p0, cnt in ramp:
                ramp_tts[p0] = [emit_ramp_half(p0, cnt, 0)]
            emit_prods(1)
            for p0, cnt in ramp:
                ramp_tts[p0].append(emit_ramp_half(p0, cnt, 1))
                emit_ramp_matmuls(p0, cnt, ramp_tts[p0])
            # first steady block is smaller so its adds finish sooner after
            # the ramp; the rest are PBLK rows
            p0 = 8
            if n_rows - p0 >= 8 and (n_rows - p0 - 8) % PBLK == 0:
                emit_block(p0, 8)
                p0 += 8
            full, last = divmod(n_rows - p0, PBLK)
            for _ in range(full):
                emit_block(p0, PBLK)
                p0 += PBLK
            if last:
                emit_block(p0, last)

            # ---- softmax over p (denominator shared across the core pair) ----
            et = [cp.tile([P, PHALF], f32, tag=f"et_{qc}", name=f"et_{qc}") for qc in range(NQC)]
            zl = cp.tile([P, NQC], f32, tag="zl")
            for qc in range(NQC):
                nc.scalar.activation(et[qc][:], st[qc][:], Act.Exp)
                nc.vector.tensor_reduce(
                    zl[:, qc : qc + 1], et[qc][:], mybir.AxisListType.X, Alu.add
                )

            zin = dramp.tile([P, NQC], f32)
            zout = dramp.tile([P, NQC], f32)
            nc.sync.dma_start(zin[:], zl[:])
            if bench_mode:
                nc.sync.dma_start(zout[:], zin[:])
            else:
                nc.gpsimd.collective_compute(
                    "AllReduce",
                    mybir.AluOpType.add,
                    replica_groups=[[0, 1], [2, 3], [4, 5], [6, 7]],
                    ins=[zin.opt()],
                    outs=[zout.opt()],
                )

            zg = cp.tile([P, NQC], f32, tag="zg")
            nc.sync.dma_start(zg[:], zout[:])
            rz = cp.tile([P, NQC], f32, tag="rz")
            nc.vector.reciprocal(rz[:], zg[:])
            ets = [cp.tile([P, PHALF], f16, tag=f"ets_{qc}", name=f"ets_{qc}") for qc in range(NQC)]
            for qc in range(NQC):
                nc.vector.tensor_scalar(
                    ets[qc][:], et[qc][:], rz[:, qc : qc + 1], None, Alu.mult
                )

            # ---- out[p, d] = sum_q (E/Z)[q, p] * q[q, d] ----
            for mc in range(NPC):
                ops = psm.tile([P, D], f32, tag="prod", name="ops", bufs=2)
                for qc in range(NQC):
                    nc.tensor.matmul(
                        ops[:],
                        ets[qc][:, mc * P : (mc + 1) * P],
                        qnf16[qc][:],
                        start=(qc == 0),
                        stop=(qc == NQC - 1),
                    )
                osb = cp.tile([P, D], f32, tag=f"osb_{mc}")
                nc.scalar.copy(osb[:], ops[:])
                dma_engines[mc % 2].dma_start(y[mc * P : (mc + 1) * P, :], osb[:])

    nc.compile()
    return nc


def _get_nc():
    if "nc" not in _cache:
        _cache["nc"] = _build()
    return _cache["nc"]


def kernel(q, p, W0, W1, vc, _trace=False, _trace_kwargs=None):
    q = np.ascontiguousarray(q, dtype=np.float32)
    p = np.ascontiguousarray(p, dtype=np.float32)
    W0 = np.ascontiguousarray(W0, dtype=np.float32)
    W1 = np.ascontiguousarray(W1, dtype=np.float32)
    vc = np.ascontiguousarray(vc, dtype=np.float32)
    eye = np.eye(P, dtype=np.float32)

    nc = _get_nc()
    from concourse.bass_utils import run_bass_kernel_spmd

    in_maps = []
    for c in range(N_CORES):
        b = c // 2
        p0 = PHALF * (c % 2)
        in_maps.append(
            {
                "qb": q[b],
                "pb": np.ascontiguousarray(p[b, p0 : p0 + PHALF]),
                "W0": W0,
                "W1": W1,
                "vc": vc,
                "eye": eye,
            }
        )

    kw = {}
    if _trace:
        kw["trace"] = True
        kw.update(_trace_kwargs or {})
    # the axon tunnel occasionally drops with a transient UNAVAILABLE
    # ("worker hung up"); retry a few times before giving up
    last_exc = None
    for attempt in range(4):
        try:
            res = run_bass_kernel_spmd(nc, in_maps, list(range(N_CORES)), **kw)
            break
        except Exception as e:  # noqa: BLE001
            last_exc = e
            if attempt == 3:
                raise
            import time as _time

            _time.sleep(5 * (attempt + 1))

    out = np.empty((B, TP, D), dtype=np.float32)
    for c in range(N_CORES):
        b = c // 2
        p0 = PHALF * (c % 2)
        out[b, p0 : p0 + PHALF] = res.results[c]["y"]

    if _trace:
        _cache["last_result"] = res
    return out


# revision 29
# speedup vs baseline: 1.0126x; 1.0006x over previous
"""Trainium2 Bass kernel for additive (Bahdanau-style) attention.

reference math (B=4, Tq=Tp=512, D=256):
    prod_q = q @ W0                                   [B,Tq,D]
    prod_p = p @ W1                                   [B,Tp,D]
    scores[b,p,q] = sum_e vc[e] * tanh(prod_p[b,p,e] + prod_q[b,q,e])
    weights = softmax(scores, axis=p)
    out[b,p,d] = sum_q weights[b,p,q] * q[b,q,d]

Sharding: 8 cores; core c handles batch b = c//2 and p-rows
[256*(c%2), 256*(c%2)+256).  The softmax denominator (per (b,q)) needs the
exp-sum over all p, so the two cores sharing a batch AllReduce a 512-float
vector; everything else is core-local.

Per-core layout: e (=D) lives on SBUF partitions (2 halves of 128).  The
broadcast add prod_p[:,p] + prod_q is a DVE tensor_scalar with a
per-partition scalar (fp32, 2x mode); tanh runs as one big ACT
instruction per p-block (fp16 out); the vc contraction is a PE matmul
(lhsT = tanh tile [e,q-chunk] fp16, rhs = vc [e,1]) accumulating score
columns S^T[q,p] in PSUM, which makes the softmax a free-axis op and
feeds the final matmul out = E^T @ (q/Z).

The kernel is ACT-bound: 256 p x 512 q x 256 e = 33.5M tanh per core at
128 lanes x 1.2 GHz = 218.5 us floor.  Cost-model timeline: ~259.5 us
per core (ACT 87% busy, gap-free through the main loop), plus the real
pairwise AllReduce (~10-20 us, not modeled).  Measured end-to-end
relative error vs the fp32 reference: 1.8e-4.
"""

import sys

if "/opt/trn_rl_repo" not in sys.path:
    sys.path.insert(0, "/opt/trn_rl_repo")

import numpy as np

B, TQ, TP, D = 4, 512, 512, 256
N_CORES = 8
PHALF = TP // 2  # p-rows per core
PBLK = 10        # p-rows per inner block
NBLK = 32
P = 128          # SBUF partitions

_cache = {}


def _build(bench_mode=False, n_blocks=NBLK):
    import concourse.bacc as bacc
    import concourse.tile as tile
    from concourse import mybir

    f32 = mybir.dt.float32
    f16 = mybir.dt.float16
    Alu = mybir.AluOpType
    Act = mybir.ActivationFunctionType

    nc = bacc.Bacc(
        "TRN2", target_bir_lowering=False, debug=False,
        num_devices=1 if bench_mode else N_CORES,
    )

    qb = nc.dram_tensor("qb", [TQ, D], f32, kind="ExternalInput")
    pb = nc.dram_tensor("pb", [PHALF, D], f32, kind="ExternalInput")
    w0 = nc.dram_tensor("W0", [D, D], f32, kind="ExternalInput")
    w1 = nc.dram_tensor("W1", [D, D], f32, kind="ExternalInput")
    vc = nc.dram_tensor("vc", [D, 1], f32, kind="ExternalInput")
    eye = nc.dram_tensor("eye", [P, P], f32, kind="ExternalInput")
    y = nc.dram_tensor("y", [PHALF, D], f32, kind="ExternalOutput")

    NQC = TQ // P   # 4 q chunks
    NDC = D // P    # 2 d/e chunks
    NPC = PHALF // P  # 2 p chunks

    with tile.TileContext(nc) as tc:
        with (
            tc.tile_pool(name="const", bufs=1) as cp,
            tc.tile_pool(name="ein", bufs=2) as einp,
            tc.tile_pool(name="tt", bufs=2) as ttp,
            tc.tile_pool(name="ps_misc", bufs=1, space="PSUM") as psm,
            tc.tile_pool(name="ps_st", bufs=1, space="PSUM") as psst,
            tc.tile_pool(name="dram", bufs=1, space="DRAM") as dramp,
        ):
            # input DMAs: one consolidated transfer per tensor (issue cost
            # on the DMA queues dominates, so fewer+larger is better), spread
            # over the two queues; qb first, it heads the critical path
            qn = cp.tile([P, NQC, D], f32, tag="qn")
            nc.sync.dma_start(
                qn[:], qb.rearrange("(c p) d -> p c d", p=P)
            )
            qn32 = [qn[:, qc, :] for qc in range(NQC)]
            w0t = cp.tile([P, NDC, D], f32, tag="w0t")
            nc.gpsimd.dma_start(
                w0t[:], w0.rearrange("(c p) d -> p c d", p=P)
            )
            eyesb = cp.tile([P, P], f32, tag="eye")
            nc.sync.dma_start(eyesb[:], eye[:])
            pn = cp.tile([P, NPC, D], f32, tag="pn")
            nc.sync.dma_start(
                pn[:], pb.rearrange("(c p) d -> p c d", p=P)
            )
            pn32 = [pn[:, pc, :] for pc in range(NPC)]
            w1t = cp.tile([P, NDC, D], f32, tag="w1t")
            nc.gpsimd.dma_start(
                w1t[:], w1.rearrange("(c p) d -> p c d", p=P)
            )
            vct = cp.tile([P, NDC], f32, tag="vct")
            nc.gpsimd.dma_start(vct[:], vc.rearrange("(c p) o -> p (c o)", p=P))
            dma_engines = [nc.sync, nc.gpsimd]

            vcbf = []
            for h in range(NDC):
                tb = cp.tile([P, 1], f16, tag=f"vcbf_{h}")
                nc.vector.tensor_copy(tb[:], vct[:, h : h + 1])
                vcbf.append(tb)

            qn16 = cp.tile([P, NQC, D], f16, tag="qn16")
            nc.vector.tensor_copy(qn16[:], qn[:])
            qnf16 = [qn16[:, qc, :] for qc in range(NQC)]
            pn16 = cp.tile([P, NPC, D], f16, tag="pn16")
            nc.vector.tensor_copy(pn16[:], pn[:])
            pn32 = [pn16[:, pc, :] for pc in range(NPC)]
            qn32 = qnf16
            w0t16 = cp.tile([P, NDC, D], f16, tag="w0t16")
            nc.vector.tensor_copy(w0t16[:], w0t[:])
            w0sb = [[w0t16[:, dc, h * P : (h + 1) * P] for h in range(NDC)]
                    for dc in range(NDC)]
            w1t16 = cp.tile([P, NDC, D], f16, tag="w1t16")
            nc.vector.tensor_copy(w1t16[:], w1t[:])
            w1sb = [[w1t16[:, dc, h * P : (h + 1) * P] for h in range(NDC)]
                    for dc in range(NDC)]
            eye16 = cp.tile([P, P], f16, tag="eye16")
            nc.vector.tensor_copy(eye16[:], eyesb[:])

            # PE transposes: qT[d, q] and pT[d, p] (fp32)
            qT = [cp.tile([P, TQ], f16, tag=f"qT_{dc}", name=f"qT_{dc}") for dc in range(NDC)]
            pT = [cp.tile([P, PHALF], f16, tag=f"pT_{dc}", name=f"pT_{dc}") for dc in range(NDC)]
            for dc in range(NDC):
                for qc in range(NQC):
                    ps = psm.tile([P, P], f16, tag="tpT", name="ps", bufs=2)
                    nc.tensor.transpose(
                        ps[:], qn32[qc][:, dc * P : (dc + 1) * P], eye16[:]
                    )
                    nc.vector.tensor_copy(qT[dc][:, qc * P : (qc + 1) * P], ps[:])
                for pc in range(NPC):
                    ps = psm.tile([P, P], f16, tag="tpT", name="ps", bufs=2)
                    nc.tensor.transpose(
                        ps[:], pn32[pc][:, dc * P : (dc + 1) * P], eye16[:]
                    )
                    nc.vector.tensor_copy(pT[dc][:, pc * P : (pc + 1) * P], ps[:])

            # prod_qT[e, q] = (q @ W0)^T  and  prod_pT[e, p] = (p @ W1)^T (fp32)
            pq = [cp.tile([P, TQ], f32, tag=f"pq_{h}", name=f"pq_{h}") for h in range(NDC)]
            pp = [cp.tile([P, PHALF], f32, tag=f"pp_{h}", name=f"pp_{h}") for h in range(NDC)]

            def emit_prods(h):
                ps = psm.tile([P, TQ], f32, tag="prod", name="ps", bufs=2)
                for dc in range(NDC):
                    nc.tensor.matmul(
                        ps[:], w0sb[dc][h][:], qT[dc][:],
                        start=(dc == 0), stop=(dc == NDC - 1),
                    )
                nc.vector.tensor_copy(pq[h][:], ps[:])
                ps2 = psm.tile([P, PHALF], f32, tag="prod", name="ps2", bufs=2)
                for dc in range(NDC):
                    nc.tensor.matmul(
                        ps2[:], w1sb[dc][h][:], pT[dc][:],
                        start=(dc == 0), stop=(dc == NDC - 1),
                    )
                nc.vector.tensor_copy(pp[h][:], ps2[:])

            # score accumulators S^T[q, p] in PSUM (fp32), one per q-chunk
            st = [psst.tile([P, PHALF], f32, tag=f"st_{qc}", name=f"st_{qc}") for qc in range(NQC)]

            # ---- main loop over p blocks ----
            # ramp-in: small h-split blocks, emitted h=0-first so the first
            # tanh only waits on the h=0 prods; then steady blocks of PBLK
            def emit_vc_matmuls(tt_ap, base_off, p0, cnt, h_list):
                for j in range(cnt):
                    pidx = p0 + j
                    for qc in range(NQC):
                        for h in h_list:
                            off = base_off(h) + j * TQ + qc * P
                            nc.tensor.matmul(
                                st[qc][:, pidx : pidx + 1],
                                tt_ap[:, off : off + P],
                                vcbf[h][:],
                                start=(h == 0),
                                stop=(h == NDC - 1),
                                skip_group_check=True,
                            )

            def emit_ramp_half(p0, cnt, h):
                # tanh for one e-half of a ramp block; matmuls are emitted
                # later (per-column h0/h1 adjacency keeps PSUM has_written
                # accumulation valid: each column's start=True must
                # immediately precede its stop=True partner on the bank)
                w = cnt * TQ
                ein = einp.tile(
                    [P, w], f32, tag=f"ein_r{p0}", name="ein", bufs=1
                )
                for j in range(cnt):
                    nc.vector.tensor_scalar(
                        ein[:, j * TQ : (j + 1) * TQ],
                        pq[h][:],
                        pp[h][:, p0 + j : p0 + j + 1],
                        None,
                        Alu.add,
                    )
                tth = ttp.tile(
                    [P, w], f16, tag=f"tt_r{p0}_{h}", name="tt", bufs=1
                )
                nc.scalar.activation(tth[:], ein[:], Act.Tanh)
                return tth

            def emit_ramp_matmuls(p0, cnt, tths):
                for j in range(cnt):
                    pidx = p0 + j
                    for qc in range(NQC):
                        for h in range(NDC):
                            off = j * TQ + qc * P
                            nc.tensor.matmul(
                                st[qc][:, pidx : pidx + 1],
                                tths[h][:, off : off + P],
                                vcbf[h][:],
                                start=(h == 0),
                                stop=(h == NDC - 1),
                                skip_group_check=True,
                            )

            def emit_block(p0, cnt):
                w = cnt * TQ
                ein = einp.tile([P, 2 * w], f32, tag="ein", name="ein")
                for h in range(NDC):
                    for j in range(cnt):
                        nc.vector.tensor_scalar(
                            ein[:, h * w + j * TQ : h * w + (j + 1) * TQ],
                            pq[h][:],
                            pp[h][:, p0 + j : p0 + j + 1],
                            None,
                            Alu.add,
                        )
                tt = ttp.tile([P, 2 * w], f16, tag="tt", name="tt")
                nc.scalar.activation(tt[:], ein[:], Act.Tanh)
                emit_vc_matmuls(tt, lambda h: h * w, p0, cnt, list(range(NDC)))

            n_rows = PHALF if n_blocks == NBLK else n_blocks * 8
            ramp = [(0, 2), (2, 6)]
            ramp_tts = {}
            emit_prods(0)
            for p0, cnt in ramp:
                ramp_tts[p0] = [emit_ramp_half(p0, cnt, 0)]
            emit_prods(1)
            for p0, cnt in ramp:
                ramp_tts[p0].append(emit_ramp_half(p0, cnt, 1))
                emit_ramp_matmuls(p0, cnt, ramp_tts[p0])
            # first steady block is smaller so its adds finish sooner after
            # the ramp; the rest are PBLK rows
            p0 = 8
            if n_rows - p0 >= 6:
                emit_block(p0, 6)
                p0 += 6
            full, last = divmod(n_rows - p0, PBLK)
            for _ in range(full):
                emit_block(p0, PBLK)
                p0 += PBLK
            if last:
                emit_block(p0, last)

            # ---- softmax over p (denominator shared across the core pair) ----
            et = [cp.tile([P, PHALF], f32, tag=f"et_{qc}", name=f"et_{qc}") for qc in range(NQC)]
            zl = cp.tile([P, NQC], f32, tag="zl")
            for qc in range(NQC):
                nc.scalar.activation(et[qc][:], st[qc][:], Act.Exp)
                nc.vector.tensor_reduce(
                    zl[:, qc : qc + 1], et[qc][:], mybir.AxisListType.X, Alu.add
                )

            zin = dramp.tile([P, NQC], f32)
            zout = dramp.tile([P, NQC], f32)
            nc.sync.dma_start(zin[:], zl[:])
            if bench_mode:
                nc.sync.dma_start(zout[:], zin[:])
            else:
                nc.gpsimd.collective_compute(
                    "AllReduce",
                    mybir.AluOpType.add,
                    replica_groups=[[0, 1], [2, 3], [4, 5], [6, 7]],
                    ins=[zin.opt()],
                    outs=[zout.opt()],
                )

            zg = cp.tile([P, NQC], f32, tag="zg")
            nc.sync.dma_start(zg[:], zout[:])
            rz = cp.tile([P, NQC], f32, tag="rz")
            nc.vector.reciprocal(rz[:], zg[:])
            ets = [cp.tile([P, PHALF], f16, tag=f"ets_{qc}", name=f"ets_{qc}") for qc in range(NQC)]
            for qc in range(NQC):
                nc.vector.tensor_scalar(
                    ets[qc][:], et[qc][:], rz[:, qc : qc + 1], None, Alu.mult
                )

            # ---- out[p, d] = sum_q (E/Z)[q, p] * q[q, d] ----
            for mc in range(NPC):
                ops = psm.tile([P, D], f32, tag="prod", name="ops", bufs=2)
                for qc in range(NQC):
                    nc.tensor.matmul(
                        ops[:],
                        ets[qc][:, mc * P : (mc + 1) * P],
                        qnf16[qc][:],
                        start=(qc == 0),
                        stop=(qc == NQC - 1),
                    )
                osb = cp.tile([P, D], f32, tag=f"osb_{mc}")
                nc.scalar.copy(osb[:], ops[:])
                dma_engines[mc % 2].dma_start(y[mc * P : (mc + 1) * P, :], osb[:])

    nc.compile()
    return nc


def _get_nc():
    if "nc" not in _cache:
        _cache["nc"] = _build()
    return _cache["nc"]


def kernel(q, p, W0, W1, vc, _trace=False, _trace_kwargs=None):
    q = np.ascontiguousarray(q, dtype=np.float32)
    p = np.ascontiguousarray(p, dtype=np.float32)
    W0 = np.ascontiguousarray(W0, dtype=np.float32)
    W1 = np.ascontiguousarray(W1, dtype=np.float32)
    vc = np.ascontiguousarray(vc, dtype=np.float32)
    eye = np.eye(P, dtype=np.float32)

    nc = _get_nc()
    from concourse.bass_utils import run_bass_kernel_spmd

    in_maps = []
    for c in range(N_CORES):
        b = c // 2
        p0 = PHALF * (c % 2)
        in_maps.append(
            {
                "qb": q[b],
                "pb": np.ascontiguousarray(p[b, p0 : p0 + PHALF]),
                "W0": W0,
                "W1": W1,
                "vc": vc,
                "eye": eye,
            }
        )

    kw = {}
    if _trace:
        kw["trace"] = True
        kw.update(_trace_kwargs or {})
    # the axon tunnel occasionally drops with a transient UNAVAILABLE
    # ("worker hung up"); retry a few times before giving up
    last_exc = None
    for attempt in range(4):
        try:
            res = run_bass_kernel_spmd(nc, in_maps, list(range(N_CORES)), **kw)
            break
        except Exception as e:  # noqa: BLE001
            last_exc = e
            if attempt == 3:
                raise
            import time as _time

            _time.sleep(5 * (attempt + 1))

    out = np.empty((B, TP, D), dtype=np.float32)
    for c in range(N_CORES):
        b = c // 2
        p0 = PHALF * (c % 2)
        out[b, p0 : p0 + PHALF] = res.results[c]["y"]

    if _trace:
        _cache["last_result"] = res
    return out
